# revision 8
# baseline (speedup 1.0000x reference)
"""Space-to-depth (8x8 chessboard) kernel for Trainium2.

Full input  : (32, 256, 256, 32) f32
Full output : (32, 8, 8, 32768) f32
out[b, i, j] = inputs[b, i*32:(i+1)*32, j*32:(j+1)*32, :].reshape(-1)

Sharding: batch dim (32) split across 8 NeuronCores (pure data parallel,
no communication) -> 4 examples per core.

Per core the op is pure HBM->HBM data movement, done entirely with DMA
access patterns (no compute engines). Key layout fact: within one
(example b, 32-row band i), iterating (r, j, elem) makes the source AP
contiguous and the destination a 3D AP, so one DMA moves a half-band
(16 rows = 512 KiB) in 4 KiB contiguous chunks:

  src [[8192, 16], [1024, 8], [1, 1024]]   (contiguous 32 KiB per row r)
  dst [[1024, 16], [32768, 8], [1, 1024]]  (4 KiB chunks, 32 KiB stride)

(DMA APs support at most 3 dims and descriptors must be contiguous on
both sides, so 4 KiB descriptors with dst-scatter is the only affine
decomposition that sprays all 16 SDMA engines; engine = outer index
mod 16.)

Performance notes (measured on trn2 via NTFF traces):
- Steady state is HBM-stack-bound: ~330 GB/s payload = ~660 GB/s
  read+write traffic out of the ~716 GB/s stack limit. Engines execute
  descriptors at line rate (158 ns / 4 KiB) alternating between the two
  HWDGE queues, with ~5% long stalls from HBM backpressure.
- The kernel tail is set by the slowest engine's finish time: keep
  per-engine totals equal (all jobs outer=16). Engine 15 is mildly
  slower (~1.5-2%) on this part, so it gets a ~3% haircut (two nr=15
  jobs; the orphan rows go to engines 0-1 via an outer=2 job).
- Sync's queue starts draining ~0.5-1.5 us before Scalar's, so Sync
  carries +0.125 MiB so both queues finish together.
- First jobs per queue are small (128 KiB) so descriptor generation for
  the first packets is fast and all 16 engines ramp together.
- Keep HWDGE DMAs at <= 128 descriptors: bigger DMAs hit a slow
  descriptor-generation fallback (10-100 us per instruction).
"""

import numpy as np

_B_PER_CORE = 4
_N_CORES = 8
_IN_SHAPE = (_B_PER_CORE, 256, 256, 32)
_OUT_SHAPE = (_B_PER_CORE, 8, 8, 32768)
_EX = 256 * 256 * 32      # elements per example  (2097152)
_BAND = 32 * 256 * 32     # elements per (example, row-band)  (262144)

_CACHE = {}


def build_nc():
    import concourse.bass as bass
    import concourse.mybir as mybir

    nc = bass.Bass(
        target_bir_lowering=False,
        enable_partition_id=False,
        monotonic_sem_count=0,
    )
    x = nc.dram_tensor("x", list(_IN_SHAPE), mybir.dt.float32, kind="ExternalInput")
    y = nc.dram_tensor("y", list(_OUT_SHAPE), mybir.dt.float32, kind="ExternalOutput")

    # Job = (b, i, r0, nr, j0, m): rows [r0, r0+nr) of band (b, i),
    # column-cells [j0, j0+m). nr=16 -> engines 0..15 get m descriptors
    # of 4 KiB each.
    def band_off(b, i):
        return b * _EX + i * _BAND

    # Band (0,0,h=0) split into 4 warmup jobs of m=2 (128 KiB).
    warm = [(0, 0, 0, 16, j0, 2) for j0 in (0, 2, 4, 6)]
    # Band (1,4,h=0) split m=5 / m=3 for queue byte-balancing.
    m5 = (1, 4, 0, 16, 0, 5)
    m3 = (1, 4, 0, 16, 5, 3)
    # Band (3,7): nr=15 jobs (engine-15 haircut). Orphan rows 15 and 31
    # are covered by the outer=2 job below (engines 0-1).
    nr15 = [(3, 7, 0, 15, 0, 8), (3, 7, 16, 15, 0, 8)]

    fulls = [
        (b, i, h * 16, 16, 0, 8)
        for b in range(_B_PER_CORE)
        for i in range(8)
        for h in range(2)
        if not ((b == 0 and i == 0 and h == 0)
                or (b == 1 and i == 4 and h == 0)
                or (b == 3 and i == 7))
    ]
    assert len(fulls) == 60

    # Sync: 2 warmups + 29 fulls + m5 + 2 nr15 + orphan  = 16.0625 MiB
    # Scalar: 2 warmups + 31 fulls + m3                   = 15.9375 MiB
    sync_fulls, scalar_fulls = fulls[0::2], fulls[1::2]
    # move one full from sync to scalar -> 29 / 31
    scalar_fulls.append(sync_fulls.pop(10))

    sync_jobs = warm[:2] + sync_fulls[:20] + [m5] + sync_fulls[20:] + nr15 + ["orph"]
    scalar_jobs = warm[2:] + scalar_fulls[:21] + [m3] + scalar_fulls[21:]

    def issue(engine, my_jobs, sem):
        n = 0
        for job in my_jobs:
            if job == "orph":
                # rows 15 and 31 of band (3,7): outer=2 -> engines 0-1
                off = band_off(3, 7)
                src = bass.AP(
                    x, off + 15 * 8192, [[16 * 8192, 2], [1024, 8], [1, 1024]]
                )
                dst = bass.AP(
                    y, off + 15 * 1024, [[16 * 1024, 2], [32768, 8], [1, 1024]]
                )
            else:
                b, i, r0, nr, j0, m = job
                off = band_off(b, i)
                src = bass.AP(
                    x, off + r0 * 8192 + j0 * 1024,
                    [[8192, nr], [1024, m], [1, 1024]],
                )
                dst = bass.AP(
                    y, off + r0 * 1024 + j0 * 32768,
                    [[1024, nr], [32768, m], [1, 1024]],
                )
            engine.dma_start(out=dst, in_=src).then_inc(sem, 16)
            n += 16
        # The final wait_ge IS the completion guarantee: Block-exit
        # DRAIN does NOT wait for outstanding HWDGE DMAs (verified on
        # HW: without this wait the program's NOTIFY fires ~20 us
        # before the last writes land).
        if n:
            engine.wait_ge(sem, n)

    with (
        nc.semaphore("sp_sem") as sp_sem,
        nc.semaphore("act_sem") as act_sem,
        nc.Block(no_gpsimd_drain=True) as block,
    ):

        @block.sync
        def _(sync):
            issue(sync, sync_jobs, sp_sem)

        @block.scalar
        def _(scalar):
            issue(scalar, scalar_jobs, act_sem)

    return nc


def _get_nc():
    if "nc" not in _CACHE:
        _CACHE["nc"] = build_nc()
    return _CACHE["nc"]


def kernel(inputs: np.ndarray) -> np.ndarray:
    from concourse.bass_utils import run_bass_kernel_spmd

    inputs = np.ascontiguousarray(np.asarray(inputs, dtype=np.float32))
    assert inputs.shape == (_B_PER_CORE * _N_CORES,) + _IN_SHAPE[1:]

    nc = _get_nc()
    in_maps = [
        {"x": np.ascontiguousarray(inputs[c * _B_PER_CORE : (c + 1) * _B_PER_CORE])}
        for c in range(_N_CORES)
    ]
    res = run_bass_kernel_spmd(nc, in_maps, core_ids=list(range(_N_CORES)))
    return np.concatenate([r["y"] for r in res.results], axis=0)
